# revision 6
# baseline (speedup 1.0000x reference)
"""Multi-head attention Trainium2 Bass kernel (v2: bf16 + flipped attn@V).

Problem: B=4, N=M=2048, DM=512, H=8, DH=64, DO=512, fp32, rel-err gate 2e-2.

Sharding: 8 cores = (batch b, head-half hh) -- each core computes heads
[4hh, 4hh+4) for all 2048 query rows of one batch, producing a partial
output [2048, 512]; the host sums the two partials per batch and adds the
constant row (sum_h v_bias_h @ Wp_h + proj_bias).

Per-core dataflow (all matmuls bf16/f32r at 1 cycle/row in the cost model):
  - x (q/k/v) arrive bf16; DMA-crossbar transpose loads xT chunks directly
    (dma_start_transpose, no PE transposes / no PSUM staging copies)
  - qTf/kTf = W^T xT  [hdh-chunk, n] f32r (bias fused into the PSUM->SBUF copy)
  - vha = xvT^T Wv    [m-tile, 4*65] bf16 (65th col = ones for softmax sums)
  - scores sc[m-tile, n-block] = kT^T qT per head (f32 PSUM)
  - exp split: ScalarE activation Exp (scale=1/8) -> bf16, and DVE via
    Schraudolph: int16 = round(sc*(128*log2e/8) + 128*(127-C)) == bf16 bits
  - attn@V flipped: oh[n-chunk, 65] = sum_mt ex[:, chunk].T @ vha[mt, h]
    (ex is the stationary operand; out free size 65 halves attn@V PE cost;
    col 64 accumulates the softmax denominator)
  - normalize: rr = recip(oh[:, :, 64]) (fast DVE approx), one broadcast-AP
    multiply -> mh bf16 (per-partition scalars, no PE broadcast)
  - mh pairs -> DMA-crossbar SBUF->SBUF transpose -> mhT [2*64, n-tile]
  - out partial[n-tile, 512] = sum_g mhT_g^T @ wp2_g (head-pair packed)
"""
import os
import sys

sys.path.insert(0, "/opt/trn_rl_repo")

import numpy as np
import ml_dtypes

import concourse.bass as bass
import concourse.mybir as mybir
import concourse.tile as tile
from concourse import bacc
from concourse.bass import AP
from concourse.bass_utils import run_bass_kernel_spmd

F32 = mybir.dt.float32
F32R = mybir.dt.float32r
BF16 = mybir.dt.bfloat16
I16 = mybir.dt.int16
EXP = mybir.ActivationFunctionType.Exp
ADD = mybir.AluOpType.add
MULT = mybir.AluOpType.mult

P = 128
DM = 512          # model dim
HC = 4            # heads per core
DH = 64
HDH = 256         # hdh per core
N = 2048          # query rows per core
M = 2048          # kv rows
DO = 512
N_MT = M // P     # 16
N_NT = N // P     # 16

SCHRAUD_C = 0.0434
A_S = float(np.float32(128.0 / np.log(2.0)) / 8.0)
B_S = float(np.float32(128.0 * (127.0 - SCHRAUD_C)))
DVE_FRAC_NUM = 7   # of 16 exp units go to DVE (Schraudolph)

_CACHED = {}
LAST_EXEC_NS = None


def _build():
    nc = bacc.Bacc("TRN2", target_bir_lowering=False, debug=False)

    d_q = nc.declare_dram_parameter("q", [N, DM], BF16, isOutput=False)
    d_k = nc.declare_dram_parameter("k", [M, DM], BF16, isOutput=False)
    d_v = nc.declare_dram_parameter("v", [M, DM], BF16, isOutput=False)
    d_wq = nc.declare_dram_parameter("wq", [DM, HDH], BF16, isOutput=False)
    d_wk = nc.declare_dram_parameter("wk", [DM, HDH], BF16, isOutput=False)
    d_wv = nc.declare_dram_parameter("wv", [DM, HDH], BF16, isOutput=False)
    d_wp = nc.declare_dram_parameter("wp", [HDH, DO], BF16, isOutput=False)
    d_qb = nc.declare_dram_parameter("qb", [P, 2], F32, isOutput=False)
    d_kb = nc.declare_dram_parameter("kb", [P, 2], F32, isOutput=False)
    d_out = nc.declare_dram_parameter("out", [N, DO], F32, isOutput=True)

    with tile.TileContext(nc) as tc:
        from contextlib import ExitStack
        with ExitStack() as ctx:
            persist = ctx.enter_context(tc.tile_pool(name="persist", bufs=1))
            ex_pool = ctx.enter_context(tc.tile_pool(name="expp", bufs=6))
            nm = ctx.enter_context(tc.tile_pool(name="nm", bufs=2))
            ot_pool = ctx.enter_context(tc.tile_pool(name="otp", bufs=2))
            ps_pp = ctx.enter_context(tc.tile_pool(name="pp", bufs=2, space="PSUM"))
            ps_sc = ctx.enter_context(tc.tile_pool(name="sc", bufs=2, space="PSUM"))
            ps_oh = ctx.enter_context(tc.tile_pool(name="oh", bufs=2, space="PSUM"))

            # --- crossbar-transposed activation loads (K, Q, V order) ---
            xkT = persist.tile([P, 4, M], BF16, tag="xkT", name="xkT")
            xqT = persist.tile([P, 4, N], BF16, tag="xqT", name="xqT")
            xvT = persist.tile([P, 4, M], BF16, tag="xvT", name="xvT")
            for dc in range(4):
                nc.sync.dma_start_transpose(
                    xkT[:, dc, :], d_k[:, dc * P:(dc + 1) * P])
            wk_sb = persist.tile([P, 4, HDH], BF16, tag="wk", name="wk")
            kb_sb = persist.tile([P, 2], F32, tag="kb", name="kb")
            for dc in range(4):
                nc.gpsimd.dma_start(wk_sb[:, dc, :], d_wk[dc * P:(dc + 1) * P, :])
            nc.gpsimd.dma_start(kb_sb[:], d_kb[:])
            for dc in range(4):
                nc.sync.dma_start_transpose(
                    xqT[:, dc, :], d_q[:, dc * P:(dc + 1) * P])
            wq_sb = persist.tile([P, 4, HDH], BF16, tag="wq", name="wq")
            qb_sb = persist.tile([P, 2], F32, tag="qb", name="qb")
            for dc in range(4):
                nc.gpsimd.dma_start(wq_sb[:, dc, :], d_wq[dc * P:(dc + 1) * P, :])
            nc.gpsimd.dma_start(qb_sb[:], d_qb[:])
            for dc in range(4):
                nc.sync.dma_start_transpose(
                    xvT[:, dc, :], d_v[:, dc * P:(dc + 1) * P])
            wv_sb = persist.tile([P, 4, HDH], BF16, tag="wv", name="wv")
            for dc in range(4):
                nc.gpsimd.dma_start(wv_sb[:, dc, :], d_wv[dc * P:(dc + 1) * P, :])
            wp_sb = persist.tile([P, 2, DO], BF16, tag="wp", name="wp")
            for g in range(2):
                nc.gpsimd.dma_start(wp_sb[:, g, :], d_wp[g * P:(g + 1) * P, :])

            # --- persistent activations ---
            kTf = persist.tile([P, 2, M], F32R, tag="kTf", name="kTf")
            qTf = persist.tile([P, 2, N], F32R, tag="qTf", name="qTf")
            vha = persist.tile([P, N_MT, HC * 65], BF16, tag="vha", name="vha")
            vhav = vha[:].rearrange("p a (h c) -> p a h c", c=65)
            mh = persist.tile([P, 2, N_NT, P], BF16, tag="mh", name="mh")
            mhT = persist.tile([P, 2, N_NT, P], BF16, tag="mhT", name="mhT")

            nc.gpsimd.memset(vhav[:, :, :, 64:65], 1.0)
            zrow = persist.tile([1, 512], BF16, tag="zrow", name="zrow")
            nc.gpsimd.memset(zrow[:], 0.0)

            # --- K / Q projections: [hdh-chunk, rows] f32r + fused bias ---
            def proj_T(xT, w_sb, b_sb, dst, nblocks):
                for ht in range(2):
                    for mb in range(nblocks):
                        pp = ps_pp.tile([P, 512], F32, tag="pp", name="pp")
                        for dc in range(4):
                            nc.tensor.matmul(
                                pp[:], w_sb[:, dc, ht * P:(ht + 1) * P],
                                xT[:, dc, mb * 512:(mb + 1) * 512],
                                start=(dc == 0), stop=(dc == 3))
                        nc.vector.tensor_scalar(
                            dst[:, ht, mb * 512:(mb + 1) * 512],
                            pp[:], b_sb[:, ht:ht + 1], None, ADD)

            proj_T(xkT, wk_sb, kb_sb, kTf, 4)
            proj_T(xqT, wq_sb, qb_sb, qTf, 4)

            # --- V projection: [m-tile, hdh] bf16 into vha ---
            for mt in range(N_MT):
                pp = ps_pp.tile([P, 512], F32, tag="pp", name="pp")
                for dc in range(4):
                    nc.tensor.matmul(
                        pp[:, 0:HDH], xvT[:, dc, mt * P:(mt + 1) * P],
                        wv_sb[:, dc, :], start=(dc == 0), stop=(dc == 3))
                nc.vector.tensor_copy(
                    vhav[:, mt, :, 0:64],
                    pp[:, 0:HDH].rearrange("p (h c) -> p h c", h=HC))

            # --- attention ---
            exp_ctr = 0
            pending_out = []

            def emit_out_group(nb):
                for c4 in range(4):
                    nt = nb * 4 + c4
                    po = ps_pp.tile([P, DO], F32, tag="pp", name="pp")
                    for g in range(2):
                        nc.tensor.matmul(
                            po[:], mhT[:, g, nt, :], wp_sb[:, g, :],
                            start=(g == 0), stop=(g == 1))
                    ot = ot_pool.tile([P, DO], F32, tag="ot", name="ot")
                    nc.scalar.copy(ot[:], po[:])
                    nc.gpsimd.dma_start(d_out[nt * P:(nt + 1) * P, :], ot[:])

            for nb in range(4):
                for h in range(HC):
                    ht, ab = h // 2, h % 2
                    if pending_out and h == 1:
                        emit_out_group(pending_out.pop())
                    oh = ps_oh.tile([P, 4, 65], F32, tag="oh", name="oh")
                    # one bank-covering zero init: sub-bank accumulation
                    # regions must not each issue start=True (the start flag
                    # zero-marks the whole 2KB PSUM bank)
                    nc.tensor.matmul(
                        oh[:].rearrange("p a b -> p (a b)"),
                        zrow[0:1, 0:P], zrow[0:1, 0:260],
                        start=True, stop=False, skip_group_check=True)
                    for mu in range(8):
                        sc = ps_sc.tile([P, 1024], F32, tag="sc", name="sc")
                        for j in range(2):
                            mt = 2 * mu + j
                            nc.tensor.matmul(
                                sc[:, j * 512:(j + 1) * 512],
                                kTf[ab * 64:ab * 64 + 64, ht, mt * P:(mt + 1) * P],
                                qTf[ab * 64:ab * 64 + 64, ht,
                                    nb * 512:(nb + 1) * 512],
                                start=True, stop=True)
                        ex = ex_pool.tile([P, 1024], I16, tag="ex", name="ex")
                        if exp_ctr % 16 < DVE_FRAC_NUM:
                            nc.vector.tensor_scalar(
                                ex[:], sc[:], A_S, B_S, MULT, ADD)
                        else:
                            nc.scalar.activation(
                                ex[:].bitcast(BF16), sc[:], EXP, scale=0.125)
                        exp_ctr += 1
                        for j in range(2):
                            mt = 2 * mu + j
                            for c4 in range(4):
                                nc.tensor.matmul(
                                    oh[:, c4, :],
                                    ex[:, j * 512 + c4 * P:
                                       j * 512 + (c4 + 1) * P].bitcast(BF16),
                                    vhav[:, mt, h, :],
                                    start=False,
                                    stop=(mu == 7 and j == 1),
                                    skip_group_check=True)
                    # normalization: per-partition reciprocal + broadcast mult
                    from concourse.dve_ops import (
                        RECIP_APPROX_FAST_CONSTS, RECIPROCAL_APPROX_FAST)
                    _c = RECIP_APPROX_FAST_CONSTS
                    rr = nm.tile([P, 4], F32, tag="rr", name="rr")
                    nc.vector._custom_dve(
                        RECIPROCAL_APPROX_FAST, out=rr[:], in0=oh[:, :, 64],
                        s0=_c["s0"], s1=_c["s1"], imm2=_c["imm2"])
                    rap = rr[:]
                    rr_b = AP(rap.tensor, rap.offset,
                              [rap.ap[0], rap.ap[1], [0, 64]])
                    nc.vector.tensor_tensor(
                        mh[:, ht, nb * 4:(nb + 1) * 4, ab * 64:ab * 64 + 64],
                        oh[:, :, 0:64], rr_b, MULT)
                # transpose mh pairs for this nb (crossbar, SBUF->SBUF)
                for g in range(2):
                    for c4 in range(4):
                        nt = nb * 4 + c4
                        nc.sync.dma_start_transpose(
                            mhT[:, g, nt, :], mh[:, g, nt, :])
                pending_out.append(nb)

            while pending_out:
                emit_out_group(pending_out.pop())

    nc.compile()
    return nc


def kernel(query, key, value, query_kernel, key_kernel, value_kernel,
           projection_kernel, q_bias, k_bias, v_bias, projection_bias):
    query = np.asarray(query, np.float32)
    key = np.asarray(key, np.float32)
    value = np.asarray(value, np.float32)
    wq = np.asarray(query_kernel, np.float32)
    wk = np.asarray(key_kernel, np.float32)
    wv = np.asarray(value_kernel, np.float32)
    wp = np.asarray(projection_kernel, np.float32)
    qb = np.asarray(q_bias, np.float32)
    kb = np.asarray(k_bias, np.float32)
    vb = np.asarray(v_bias, np.float32)
    pb = np.asarray(projection_bias, np.float32)

    B = query.shape[0]
    const_row = (np.einsum("hi,hio->o", vb.astype(np.float64),
                           wp.astype(np.float64))
                 + pb.astype(np.float64)).astype(np.float32)

    bfq = [np.ascontiguousarray(query[b]).astype(ml_dtypes.bfloat16)
           for b in range(B)]
    bfk = [np.ascontiguousarray(key[b]).astype(ml_dtypes.bfloat16)
           for b in range(B)]
    bfv = [np.ascontiguousarray(value[b]).astype(ml_dtypes.bfloat16)
           for b in range(B)]

    halves = []
    for hh in range(2):
        hs = slice(hh * HC, (hh + 1) * HC)
        halves.append(dict(
            wq=np.ascontiguousarray(
                wq[hs].transpose(1, 0, 2).reshape(DM, HDH)).astype(
                ml_dtypes.bfloat16),
            wk=np.ascontiguousarray(
                wk[hs].transpose(1, 0, 2).reshape(DM, HDH)).astype(
                ml_dtypes.bfloat16),
            wv=np.ascontiguousarray(
                wv[hs].transpose(1, 0, 2).reshape(DM, HDH)).astype(
                ml_dtypes.bfloat16),
            wp=np.ascontiguousarray(
                wp[hs].reshape(HDH, DO)).astype(ml_dtypes.bfloat16),
            qb=np.ascontiguousarray(qb[hs].reshape(HDH).reshape(2, P).T),
            kb=np.ascontiguousarray(kb[hs].reshape(HDH).reshape(2, P).T),
        ))

    if "nc" not in _CACHED:
        _CACHED["nc"] = _build()
    nc = _CACHED["nc"]

    in_maps = []
    for c in range(8):
        b, hh = c // 2, c % 2
        in_maps.append(dict(q=bfq[b], k=bfk[b], v=bfv[b], **halves[hh]))

    trace = os.environ.get("KERNEL_TRACE", "0") == "1"
    try:
        res = run_bass_kernel_spmd(nc, in_maps, core_ids=list(range(8)),
                                   trace=trace)
    except ModuleNotFoundError:
        res = run_bass_kernel_spmd(nc, in_maps, core_ids=list(range(8)),
                                   trace=False)
    global LAST_EXEC_NS
    LAST_EXEC_NS = res.exec_time_ns
    if trace and res.exec_time_ns is not None:
        print(f"HW exec time: {res.exec_time_ns} ns")
        if res.instructions_and_trace is not None:
            print(f"trace: {res.instructions_and_trace[1]}")

    out = np.empty((B, N, DO), dtype=np.float32)
    for b in range(B):
        out[b] = (res.results[2 * b]["out"] + res.results[2 * b + 1]["out"]
                  + const_row[None, :])
    return out


# revision 19
# speedup vs baseline: 1.0893x; 1.0893x over previous
"""Multi-head attention Trainium2 Bass kernel (v2: bf16 + flipped attn@V).

Problem: B=4, N=M=2048, DM=512, H=8, DH=64, DO=512, fp32, rel-err gate 2e-2.

Sharding: 8 cores = (batch b, head-half hh) -- each core computes heads
[4hh, 4hh+4) for all 2048 query rows of one batch, producing a partial
output [2048, 512]; the host sums the two partials per batch and adds the
constant row (sum_h v_bias_h @ Wp_h + proj_bias).

Per-core dataflow (all matmuls bf16/f32r at 1 cycle/row in the cost model):
  - x (q/k/v) arrive bf16; DMA-crossbar transpose loads xT chunks directly
    (dma_start_transpose, no PE transposes / no PSUM staging copies)
  - qTf/kTf = W^T xT  [hdh-chunk, n] f32r (bias fused into the PSUM->SBUF copy)
  - vha = xvT^T Wv    [m-tile, 4*65] bf16 (65th col = ones for softmax sums)
  - scores sc[m-tile, n-block] = kT^T qT per head (f32 PSUM)
  - exp split: ScalarE activation Exp (scale=1/8) -> bf16, and DVE via
    Schraudolph: int16 = round(sc*(128*log2e/8) + 128*(127-C)) == bf16 bits
  - attn@V flipped: oh[n-chunk, 65] = sum_mt ex[:, chunk].T @ vha[mt, h]
    (ex is the stationary operand; out free size 65 halves attn@V PE cost;
    col 64 accumulates the softmax denominator)
  - normalize: rr = recip(oh[:, :, 64]) (fast DVE approx), one broadcast-AP
    multiply -> mh bf16 (per-partition scalars, no PE broadcast)
  - mh pairs -> DMA-crossbar SBUF->SBUF transpose -> mhT [2*64, n-tile]
  - out partial[n-tile, 512] = sum_g mhT_g^T @ wp2_g (head-pair packed)
"""
import os
import sys

sys.path.insert(0, "/opt/trn_rl_repo")

import numpy as np
import ml_dtypes

import concourse.bass as bass
import concourse.mybir as mybir
import concourse.tile as tile
from concourse import bacc
from concourse.bass import AP
from concourse.bass_utils import run_bass_kernel_spmd

F32 = mybir.dt.float32
F32R = mybir.dt.float32r
BF16 = mybir.dt.bfloat16
I16 = mybir.dt.int16
EXP = mybir.ActivationFunctionType.Exp
ADD = mybir.AluOpType.add
MULT = mybir.AluOpType.mult

P = 128
DM = 512          # model dim
HC = 4            # heads per core
DH = 64
HDH = 256         # hdh per core
N = 2048          # query rows per core
M = 2048          # kv rows
DO = 512
N_MT = M // P     # 16
N_NT = N // P     # 16

SCHRAUD_C = 0.0434
A_S = float(np.float32(128.0 / np.log(2.0)) / 8.0)
B_S = float(np.float32(128.0 * (127.0 - SCHRAUD_C)))
DVE_FRAC_NUM = 7   # of 16 exp units go to DVE (Schraudolph)

_CACHED = {}
LAST_EXEC_NS = None


def _build():
    nc = bacc.Bacc("TRN2", target_bir_lowering=False, debug=False)

    d_q = nc.declare_dram_parameter("q", [N, DM], BF16, isOutput=False)
    d_k = nc.declare_dram_parameter("k", [M, DM], BF16, isOutput=False)
    d_v = nc.declare_dram_parameter("v", [M, DM], BF16, isOutput=False)
    d_wq = nc.declare_dram_parameter("wq", [DM, HDH], BF16, isOutput=False)
    d_wk = nc.declare_dram_parameter("wk", [DM, HDH], BF16, isOutput=False)
    d_wv = nc.declare_dram_parameter("wv", [DM, HDH], BF16, isOutput=False)
    d_wp = nc.declare_dram_parameter("wp", [HDH, DO], BF16, isOutput=False)
    d_qb = nc.declare_dram_parameter("qb", [P, 2], F32, isOutput=False)
    d_kb = nc.declare_dram_parameter("kb", [P, 2], F32, isOutput=False)
    d_out = nc.declare_dram_parameter("out", [N, DO], F32, isOutput=True)

    with tile.TileContext(nc) as tc:
        from contextlib import ExitStack
        with ExitStack() as ctx:
            persist = ctx.enter_context(tc.tile_pool(name="persist", bufs=1))
            ex_pool = ctx.enter_context(tc.tile_pool(name="expp", bufs=6))
            nm = ctx.enter_context(tc.tile_pool(name="nm", bufs=2))
            ot_pool = ctx.enter_context(tc.tile_pool(name="otp", bufs=2))
            ps_pp = ctx.enter_context(tc.tile_pool(name="pp", bufs=2, space="PSUM"))
            ps_sc = ctx.enter_context(tc.tile_pool(name="sc", bufs=2, space="PSUM"))
            ps_oh = ctx.enter_context(tc.tile_pool(name="oh", bufs=2, space="PSUM"))

            # --- crossbar-transposed activation loads (K, Q, V order),
            # spread across both HWDGE engines (SP + Activation) ---
            xkT = persist.tile([P, 4, M], BF16, tag="xkT", name="xkT")
            xqT = persist.tile([P, 4, N], BF16, tag="xqT", name="xqT")
            xvT = persist.tile([P, 4, M], BF16, tag="xvT", name="xvT")
            hweng = [nc.sync, nc.sync]

            # --- persistent activations ---
            kTf = persist.tile([P, 2, M], F32R, tag="kTf", name="kTf")
            qTf = persist.tile([P, 2, N], F32R, tag="qTf", name="qTf")
            vha = persist.tile([P, N_MT, HC * 65], BF16, tag="vha", name="vha")
            vhav = vha[:].rearrange("p a (h c) -> p a h c", c=65)
            mh = persist.tile([P, 2, N_NT, P], BF16, tag="mh", name="mh")
            mhT = persist.tile([P, 2, N_NT, P], BF16, tag="mhT", name="mhT")
            wk_sb = persist.tile([P, 4, HDH], BF16, tag="wk", name="wk")
            kb_sb = persist.tile([P, 2], F32, tag="kb", name="kb")
            wq_sb = persist.tile([P, 4, HDH], BF16, tag="wq", name="wq")
            qb_sb = persist.tile([P, 2], F32, tag="qb", name="qb")
            wv_sb = persist.tile([P, 4, HDH], BF16, tag="wv", name="wv")
            wp_sb = persist.tile([P, 2, DO], BF16, tag="wp", name="wp")
            zrow = persist.tile([1, 512], BF16, tag="zrow", name="zrow")

            def proj_block(xT, w_sb, b_sb, dst, mb):
                for ht in range(2):
                    pp = ps_pp.tile([P, 512], F32, tag="pp", name="pp")
                    for dc in range(4):
                        nc.tensor.matmul(
                            pp[:], w_sb[:, dc, ht * P:(ht + 1) * P],
                            xT[:, dc, mb * 512:(mb + 1) * 512],
                            start=(dc == 0), stop=(dc == 3))
                    nc.vector.tensor_scalar(
                        dst[:, ht, mb * 512:(mb + 1) * 512],
                        pp[:], b_sb[:, ht:ht + 1], None, ADD)

            def vproj_tile(mt):
                pp = ps_pp.tile([P, 512], F32, tag="pp", name="pp")
                for dc in range(4):
                    nc.tensor.matmul(
                        pp[:, 0:HDH], xvT[:, dc, mt * P:(mt + 1) * P],
                        wv_sb[:, dc, :], start=(dc == 0), stop=(dc == 3))
                nc.vector.tensor_copy(
                    vhav[:, mt, :, 0:64],
                    pp[:, 0:HDH].rearrange("p (h c) -> p h c", h=HC))

            # weights + constants first (Pool/SWDGE queue, independent of
            # the HWDGE crossbar loads)
            nc.gpsimd.memset(vhav[:, :, :, 64:65], 1.0)
            nc.gpsimd.memset(zrow[:], 0.0)
            for dc in range(4):
                nc.gpsimd.dma_start(wk_sb[:, dc, :], d_wk[dc * P:(dc + 1) * P, :])
            nc.gpsimd.dma_start(kb_sb[:], d_kb[:])
            for dc in range(4):
                nc.gpsimd.dma_start(wq_sb[:, dc, :], d_wq[dc * P:(dc + 1) * P, :])
            nc.gpsimd.dma_start(qb_sb[:], d_qb[:])
            for dc in range(4):
                nc.gpsimd.dma_start(wv_sb[:, dc, :], d_wv[dc * P:(dc + 1) * P, :])
            for g in range(2):
                nc.gpsimd.dma_start(wp_sb[:, g, :], d_wp[g * P:(g + 1) * P, :])

            # crossbar loads in row-halves so compute can start after the
            # first half: K h0, Q h0, V h0, then second halves
            def xload(xT, d_x, rows, half):
                h0, h1 = half * (rows // 2), (half + 1) * (rows // 2)
                for dc in range(4):
                    hweng[dc % 2].dma_start_transpose(
                        xT[:, dc, h0:h1], d_x[h0:h1, dc * P:(dc + 1) * P])

            xload(xkT, d_k, M, 0)
            xload(xqT, d_q, N, 0)
            xload(xvT, d_v, M, 0)
            xload(xkT, d_k, M, 1)
            xload(xvT, d_v, M, 1)
            xload(xqT, d_q, N, 1)

            # K-proj m-blocks 0-1 (need only half 0) + Q-proj n-block 0;
            # the rest is interleaved into the attention stream below
            proj_block(xkT, wk_sb, kb_sb, kTf, 0)
            proj_block(xkT, wk_sb, kb_sb, kTf, 1)
            proj_block(xqT, wq_sb, qb_sb, qTf, 0)
            vproj_tile(0)
            vproj_tile(1)

            # --- attention ---
            exp_ctr = 0
            pending_out = []

            def emit_out_group(nb):
                for c4 in range(4):
                    nt = nb * 4 + c4
                    po = ps_pp.tile([P, DO], F32, tag="pp", name="pp")
                    for g in range(2):
                        nc.tensor.matmul(
                            po[:], mhT[:, g, nt, :], wp_sb[:, g, :],
                            start=(g == 0), stop=(g == 1))
                    ot = ot_pool.tile([P, DO], F32, tag="ot", name="ot")
                    if c4 % 2 == 0:
                        nc.scalar.copy(ot[:], po[:])
                    else:
                        nc.vector.tensor_copy(ot[:], po[:])
                    nc.gpsimd.dma_start(d_out[nt * P:(nt + 1) * P, :], ot[:])

            for nb in range(4):
                for h in range(HC):
                    ht, ab = h // 2, h % 2
                    if h == 0 and nb < 3:
                        proj_block(xqT, wq_sb, qb_sb, qTf, nb + 1)
                    if pending_out and h == 1:
                        emit_out_group(pending_out.pop())
                    oh = ps_oh.tile([P, 4, 65], F32, tag="oh", name="oh")
                    # one bank-covering zero init: sub-bank accumulation
                    # regions must not each issue start=True (the start flag
                    # zero-marks the whole 2KB PSUM bank)
                    nc.tensor.matmul(
                        oh[:].rearrange("p a b -> p (a b)"),
                        zrow[0:1, 0:P], zrow[0:1, 0:260],
                        start=True, stop=False, skip_group_check=True)

                    def emit_attnv(oh, h, mu, exd):
                        for j in range(2):
                            mt = 2 * mu + j
                            for c4 in range(4):
                                nc.tensor.matmul(
                                    oh[:, c4, :],
                                    exd[:, j * 512 + c4 * P:
                                        j * 512 + (c4 + 1) * P].bitcast(BF16),
                                    vhav[:, mt, h, :],
                                    start=False,
                                    stop=(mu == 7 and j == 1),
                                    skip_group_check=True)

                    SKEW = 2
                    exs = {}
                    for mu in range(8):
                        if nb == 0 and h == 0 and mu < 7:
                            if mu == 1:
                                proj_block(xkT, wk_sb, kb_sb, kTf, 2)
                            if mu == 2:
                                proj_block(xkT, wk_sb, kb_sb, kTf, 3)
                            vproj_tile(2 * mu + 2)
                            vproj_tile(2 * mu + 3)
                        sc = ps_sc.tile([P, 1024], F32, tag="sc", name="sc")
                        for j in range(2):
                            mt = 2 * mu + j
                            nc.tensor.matmul(
                                sc[:, j * 512:(j + 1) * 512],
                                kTf[ab * 64:ab * 64 + 64, ht, mt * P:(mt + 1) * P],
                                qTf[ab * 64:ab * 64 + 64, ht,
                                    nb * 512:(nb + 1) * 512],
                                start=True, stop=True)
                        ex = ex_pool.tile([P, 1024], I16, tag="ex", name="ex")
                        # interleave engines: odd slots (7 of 16) on DVE so
                        # ScalarE and DVE exps overlap instead of running in
                        # long single-engine bursts
                        if exp_ctr % 16 in (1, 3, 5, 7, 9, 11, 13):
                            nc.vector.tensor_scalar(
                                ex[:], sc[:], A_S, B_S, MULT, ADD)
                        else:
                            nc.scalar.activation(
                                ex[:].bitcast(BF16), sc[:], EXP, scale=0.125)
                        exp_ctr += 1
                        exs[mu] = ex
                        if mu >= SKEW:
                            emit_attnv(oh, h, mu - SKEW, exs.pop(mu - SKEW))
                    for mu in range(8 - SKEW, 8):
                        emit_attnv(oh, h, mu, exs.pop(mu))
                    # normalization: per-partition reciprocal + broadcast mult
                    from concourse.dve_ops import (
                        RECIP_APPROX_FAST_CONSTS, RECIPROCAL_APPROX_FAST)
                    _c = RECIP_APPROX_FAST_CONSTS
                    rr = nm.tile([P, 4], F32, tag="rr", name="rr")
                    nc.vector._custom_dve(
                        RECIPROCAL_APPROX_FAST, out=rr[:], in0=oh[:, :, 64],
                        s0=_c["s0"], s1=_c["s1"], imm2=_c["imm2"])
                    rap = rr[:]
                    rr_b = AP(rap.tensor, rap.offset,
                              [rap.ap[0], rap.ap[1], [0, 64]])
                    nc.vector.tensor_tensor(
                        mh[:, ht, nb * 4:(nb + 1) * 4, ab * 64:ab * 64 + 64],
                        oh[:, :, 0:64], rr_b, MULT)
                    if ab == 1:
                        # pair ht complete for this nb: transpose right away
                        # (crossbar SBUF->SBUF) on SP only -- the Activation
                        # SEQ must stay free for exp dispatch
                        for c4 in range(4):
                            nt = nb * 4 + c4
                            nc.sync.dma_start_transpose(
                                mhT[:, ht, nt, :], mh[:, ht, nt, :])
                if nb == 3:
                    # final block: no later head iteration will flush it
                    emit_out_group(nb)
                else:
                    pending_out.append(nb)

            while pending_out:
                emit_out_group(pending_out.pop())

    nc.compile()
    return nc


def kernel(query, key, value, query_kernel, key_kernel, value_kernel,
           projection_kernel, q_bias, k_bias, v_bias, projection_bias):
    query = np.asarray(query, np.float32)
    key = np.asarray(key, np.float32)
    value = np.asarray(value, np.float32)
    wq = np.asarray(query_kernel, np.float32)
    wk = np.asarray(key_kernel, np.float32)
    wv = np.asarray(value_kernel, np.float32)
    wp = np.asarray(projection_kernel, np.float32)
    qb = np.asarray(q_bias, np.float32)
    kb = np.asarray(k_bias, np.float32)
    vb = np.asarray(v_bias, np.float32)
    pb = np.asarray(projection_bias, np.float32)

    B = query.shape[0]
    const_row = (np.einsum("hi,hio->o", vb.astype(np.float64),
                           wp.astype(np.float64))
                 + pb.astype(np.float64)).astype(np.float32)

    bfq = [np.ascontiguousarray(query[b]).astype(ml_dtypes.bfloat16)
           for b in range(B)]
    bfk = [np.ascontiguousarray(key[b]).astype(ml_dtypes.bfloat16)
           for b in range(B)]
    bfv = [np.ascontiguousarray(value[b]).astype(ml_dtypes.bfloat16)
           for b in range(B)]

    halves = []
    for hh in range(2):
        hs = slice(hh * HC, (hh + 1) * HC)
        halves.append(dict(
            wq=np.ascontiguousarray(
                wq[hs].transpose(1, 0, 2).reshape(DM, HDH)).astype(
                ml_dtypes.bfloat16),
            wk=np.ascontiguousarray(
                wk[hs].transpose(1, 0, 2).reshape(DM, HDH)).astype(
                ml_dtypes.bfloat16),
            wv=np.ascontiguousarray(
                wv[hs].transpose(1, 0, 2).reshape(DM, HDH)).astype(
                ml_dtypes.bfloat16),
            wp=np.ascontiguousarray(
                wp[hs].reshape(HDH, DO)).astype(ml_dtypes.bfloat16),
            qb=np.ascontiguousarray(qb[hs].reshape(HDH).reshape(2, P).T),
            kb=np.ascontiguousarray(kb[hs].reshape(HDH).reshape(2, P).T),
        ))

    if "nc" not in _CACHED:
        _CACHED["nc"] = _build()
    nc = _CACHED["nc"]

    in_maps = []
    for c in range(8):
        b, hh = c // 2, c % 2
        in_maps.append(dict(q=bfq[b], k=bfk[b], v=bfv[b], **halves[hh]))

    trace = os.environ.get("KERNEL_TRACE", "0") == "1"
    try:
        res = run_bass_kernel_spmd(nc, in_maps, core_ids=list(range(8)),
                                   trace=trace)
    except ModuleNotFoundError:
        res = run_bass_kernel_spmd(nc, in_maps, core_ids=list(range(8)),
                                   trace=False)
    global LAST_EXEC_NS
    LAST_EXEC_NS = res.exec_time_ns
    if trace and res.exec_time_ns is not None:
        print(f"HW exec time: {res.exec_time_ns} ns")
        if res.instructions_and_trace is not None:
            print(f"trace: {res.instructions_and_trace[1]}")

    out = np.empty((B, N, DO), dtype=np.float32)
    for b in range(B):
        out[b] = (res.results[2 * b]["out"] + res.results[2 * b + 1]["out"]
                  + const_row[None, :])
    return out


# revision 30
# speedup vs baseline: 1.2336x; 1.1325x over previous
"""Multi-head attention Trainium2 Bass kernel (v5).

Problem: B=4, N=M=2048, DM=512, H=8, DH=64, DO=512, fp32, rel-err gate 2e-2.

Sharding: 8 cores = (batch b, head-half hh) -- each core computes heads
[4hh, 4hh+4) for all 2048 query rows of one batch, producing a partial
output [2048, 512]; the host sums the two partials per batch and adds the
constant row (sum_h v_bias_h @ Wp_h + proj_bias).

Per-core dataflow (all matmuls bf16/f32r at 1 cycle/row in the cost model):
  - k/q arrive bf16, transposed on load by the DMA crossbar (SP queue only --
    Activation-issued dma_start_transpose returns wrong data on HW);
    loads are split (K halves, Q quarter-first) so projections start early
  - v loaded untransposed in one DMA; transposed on the PE per m-tile
  - qTf/kTf = W^T xT  [hdh-chunk, n] f32r (bias fused into the PSUM copy)
  - vha = xvT^T Wv    [m-tile, 4*65] bf16 (65th col = ones -> softmax sums)
  - scores sc[m-tile, n-block] = kT^T qT per head (f32 PSUM)
  - exp split across engines (interleaved): ScalarE activation Exp
    (scale=1/8) -> bf16; DVE Schraudolph int16 = round(sc*a+b) == bf16 bits
  - attn@V flipped: oh[n-chunk, 65] += ex_chunk^T @ vha[mt, h] (ex is the
    stationary operand; out free size 65; col 64 = softmax denominator);
    oh's 4 sub-bank accumulators share one bank-covering zero-init matmul
  - normalize: rr = fast-recip(oh[:, :, 64]); one broadcast-AP multiply
  - mh pairs transposed on the PE -> mhT; out partial = sum_g mhT_g^T wp2_g
"""
import os
import sys

sys.path.insert(0, "/opt/trn_rl_repo")

import numpy as np
import ml_dtypes

import concourse.bass as bass
import concourse.mybir as mybir
import concourse.tile as tile
from concourse import bacc
from concourse.bass import AP
from concourse.bass_utils import run_bass_kernel_spmd

F32 = mybir.dt.float32
F32R = mybir.dt.float32r
BF16 = mybir.dt.bfloat16
I16 = mybir.dt.int16
EXP = mybir.ActivationFunctionType.Exp
ADD = mybir.AluOpType.add
MULT = mybir.AluOpType.mult

P = 128
DM = 512
HC = 4            # heads per core
DH = 64
HDH = 256         # hdh per core
N = 2048
M = 2048
DO = 512
N_MT = M // P
N_NT = N // P

SCHRAUD_C = 0.0434
A_S = float(np.float32(128.0 / np.log(2.0)) / 8.0)
B_S = float(np.float32(128.0 * (127.0 - SCHRAUD_C)))

_CACHED = {}
LAST_EXEC_NS = None


def _build():
    nc = bacc.Bacc("TRN2", target_bir_lowering=False, debug=False)

    d_q = nc.declare_dram_parameter("q", [N, DM], BF16, isOutput=False)
    d_k = nc.declare_dram_parameter("k", [M, DM], BF16, isOutput=False)
    d_v = nc.declare_dram_parameter("v", [M, DM], BF16, isOutput=False)
    d_wq = nc.declare_dram_parameter("wq", [DM, HDH], BF16, isOutput=False)
    d_wk = nc.declare_dram_parameter("wk", [DM, HDH], BF16, isOutput=False)
    d_wv = nc.declare_dram_parameter("wv", [DM, HDH], BF16, isOutput=False)
    d_wp = nc.declare_dram_parameter("wp", [HDH, DO], BF16, isOutput=False)
    d_qb = nc.declare_dram_parameter("qb", [P, 2], F32, isOutput=False)
    d_kb = nc.declare_dram_parameter("kb", [P, 2], F32, isOutput=False)
    d_id = nc.declare_dram_parameter("ident", [P, P], BF16, isOutput=False)
    d_out = nc.declare_dram_parameter("out", [N, DO], F32, isOutput=True)

    with tile.TileContext(nc) as tc:
        from contextlib import ExitStack
        with ExitStack() as ctx:
            persist = ctx.enter_context(tc.tile_pool(name="persist", bufs=1))
            ex_pool = ctx.enter_context(tc.tile_pool(name="expp", bufs=6))
            vtt_pool = ctx.enter_context(tc.tile_pool(name="vttp", bufs=2))
            nm = ctx.enter_context(tc.tile_pool(name="nm", bufs=2))
            ot_pool = ctx.enter_context(tc.tile_pool(name="otp", bufs=2))
            ps_pp = ctx.enter_context(tc.tile_pool(name="pp", bufs=2, space="PSUM"))
            ps_sc = ctx.enter_context(tc.tile_pool(name="sc", bufs=2, space="PSUM"))
            ps_oh = ctx.enter_context(tc.tile_pool(name="oh", bufs=2, space="PSUM"))

            xkT = persist.tile([P, 4, M], BF16, tag="xkT", name="xkT")
            xqT = persist.tile([P, 4, N], BF16, tag="xqT", name="xqT")
            v_raw = persist.tile([P, N_MT, DM], BF16, tag="v_raw", name="v_raw")
            kTf = persist.tile([P, 2, M], F32R, tag="kTf", name="kTf")
            qTf = persist.tile([P, 2, N], F32R, tag="qTf", name="qTf")
            vha = persist.tile([P, N_MT, HC * 65], BF16, tag="vha", name="vha")
            vhav = vha[:].rearrange("p a (h c) -> p a h c", c=65)
            mh = persist.tile([P, 2, N_NT, P], BF16, tag="mh", name="mh")
            mhT = persist.tile([P, 2, N_NT, P], BF16, tag="mhT", name="mhT")
            wk_sb = persist.tile([P, 4, HDH], BF16, tag="wk", name="wk")
            kb_sb = persist.tile([P, 2], F32, tag="kb", name="kb")
            wq_sb = persist.tile([P, 4, HDH], BF16, tag="wq", name="wq")
            qb_sb = persist.tile([P, 2], F32, tag="qb", name="qb")
            wv_sb = persist.tile([P, 4, HDH], BF16, tag="wv", name="wv")
            wp_sb = persist.tile([P, 2, DO], BF16, tag="wp", name="wp")
            ident = persist.tile([P, P], BF16, tag="ident", name="ident")
            zrow = persist.tile([1, 512], BF16, tag="zrow", name="zrow")

            # all loads on the SP/HWDGE queue in dependency order -- mixing
            # SWDGE (Pool) and HWDGE DMAs serializes them with multi-us
            # round-trips, while a pure HWDGE stream pipelines at ~650ns
            nc.gpsimd.memset(vhav[:, :, :, 64:65], 1.0)
            nc.gpsimd.memset(zrow[:], 0.0)

            def xload(xT, d_x, r0, r1):
                for dc in range(4):
                    nc.sync.dma_start_transpose(
                        xT[:, dc, r0:r1], d_x[r0:r1, dc * P:(dc + 1) * P])

            nc.sync.dma_start(
                wk_sb[:], d_wk[:].rearrange("(a p) c -> p a c", p=P))
            nc.sync.dma_start(kb_sb[:], d_kb[:])
            xload(xkT, d_k, 0, 1024)
            nc.sync.dma_start(
                wq_sb[:], d_wq[:].rearrange("(a p) c -> p a c", p=P))
            nc.sync.dma_start(qb_sb[:], d_qb[:])
            xload(xqT, d_q, 0, 512)
            nc.sync.dma_start(ident[:], d_id[:])
            nc.sync.dma_start(
                wv_sb[:], d_wv[:].rearrange("(a p) c -> p a c", p=P))
            nc.sync.dma_start(
                v_raw[:], d_v[:].rearrange("(a p) c -> p a c", p=P))
            xload(xkT, d_k, 1024, 2048)
            xload(xqT, d_q, 512, 2048)
            nc.sync.dma_start(
                wp_sb[:], d_wp[:].rearrange("(a p) c -> p a c", p=P))

            def proj_block(xT, w_sb, b_sb, dst, mb):
                for ht in range(2):
                    pp = ps_pp.tile([P, 512], F32, tag="pp", name="pp")
                    for dc in range(4):
                        nc.tensor.matmul(
                            pp[:], w_sb[:, dc, ht * P:(ht + 1) * P],
                            xT[:, dc, mb * 512:(mb + 1) * 512],
                            start=(dc == 0), stop=(dc == 3))
                    nc.vector.tensor_scalar(
                        dst[:, ht, mb * 512:(mb + 1) * 512],
                        pp[:], b_sb[:, ht:ht + 1], None, ADD)

            def vproj_tile(mt):
                # PE-transpose the raw v tile, then project into vha
                pst = ps_pp.tile([P, 4, P], BF16, tag="pp", name="pp")
                for dc in range(4):
                    nc.tensor.transpose(
                        pst[:, dc, :], v_raw[:, mt, dc * P:(dc + 1) * P],
                        ident[:])
                vtt = vtt_pool.tile([P, 4, P], BF16, tag="vtt", name="vtt")
                nc.scalar.copy(vtt[:], pst[:])
                pp = ps_pp.tile([P, 512], F32, tag="pp", name="pp")
                for dc in range(4):
                    nc.tensor.matmul(
                        pp[:, 0:HDH], vtt[:, dc, :],
                        wv_sb[:, dc, :], start=(dc == 0), stop=(dc == 3))
                nc.vector.tensor_copy(
                    vhav[:, mt, :, 0:64],
                    pp[:, 0:HDH].rearrange("p (h c) -> p h c", h=HC))

            proj_block(xkT, wk_sb, kb_sb, kTf, 0)
            proj_block(xkT, wk_sb, kb_sb, kTf, 1)
            proj_block(xqT, wq_sb, qb_sb, qTf, 0)
            vproj_tile(0)
            vproj_tile(1)

            # --- attention ---
            exp_ctr = 0
            pending_out = []

            def emit_out_group(nb):
                for c4 in range(4):
                    nt = nb * 4 + c4
                    po = ps_pp.tile([P, DO], F32, tag="pp", name="pp")
                    for g in range(2):
                        nc.tensor.matmul(
                            po[:], mhT[:, g, nt, :], wp_sb[:, g, :],
                            start=(g == 0), stop=(g == 1))
                    ot = ot_pool.tile([P, DO], F32, tag="ot", name="ot")
                    if c4 % 2 == 0:
                        nc.scalar.copy(ot[:], po[:])
                    else:
                        nc.vector.tensor_copy(ot[:], po[:])
                    nc.gpsimd.dma_start(d_out[nt * P:(nt + 1) * P, :], ot[:])

            for nb in range(4):
                for h in range(HC):
                    ht, ab = h // 2, h % 2
                    if h == 0 and nb < 3:
                        proj_block(xqT, wq_sb, qb_sb, qTf, nb + 1)
                    if pending_out and h == 1:
                        emit_out_group(pending_out.pop())
                    oh = ps_oh.tile([P, 4, 65], F32, tag="oh", name="oh")
                    # one bank-covering zero init: sub-bank accumulation
                    # regions must not each issue start=True (the start flag
                    # zero-marks the whole 2KB PSUM bank)
                    nc.tensor.matmul(
                        oh[:].rearrange("p a b -> p (a b)"),
                        zrow[0:1, 0:P], zrow[0:1, 0:260],
                        start=True, stop=False, skip_group_check=True)

                    def emit_attnv(oh, h, mu, exd):
                        for j in range(2):
                            mt = 2 * mu + j
                            for c4 in range(4):
                                nc.tensor.matmul(
                                    oh[:, c4, :],
                                    exd[:, j * 512 + c4 * P:
                                        j * 512 + (c4 + 1) * P].bitcast(BF16),
                                    vhav[:, mt, h, :],
                                    start=False,
                                    stop=(mu == 7 and j == 1),
                                    skip_group_check=True)

                    SKEW = 3
                    exs = {}
                    for mu in range(8):
                        if nb == 0 and h == 0 and mu < 7:
                            if mu == 1:
                                proj_block(xkT, wk_sb, kb_sb, kTf, 2)
                            if mu == 2:
                                proj_block(xkT, wk_sb, kb_sb, kTf, 3)
                            vproj_tile(2 * mu + 2)
                            vproj_tile(2 * mu + 3)
                        sc = ps_sc.tile([P, 1024], F32, tag="sc", name="sc")
                        for j in range(2):
                            mt = 2 * mu + j
                            nc.tensor.matmul(
                                sc[:, j * 512:(j + 1) * 512],
                                kTf[ab * 64:ab * 64 + 64, ht, mt * P:(mt + 1) * P],
                                qTf[ab * 64:ab * 64 + 64, ht,
                                    nb * 512:(nb + 1) * 512],
                                start=True, stop=True)
                        ex = ex_pool.tile([P, 1024], I16, tag="ex", name="ex")
                        # interleave engines (7 of 16 on DVE) so ScalarE and
                        # DVE exps overlap instead of single-engine bursts
                        if exp_ctr % 16 in (1, 3, 5, 7, 9, 11, 13):
                            nc.vector.tensor_scalar(
                                ex[:], sc[:], A_S, B_S, MULT, ADD)
                        else:
                            nc.scalar.activation(
                                ex[:].bitcast(BF16), sc[:], EXP, scale=0.125)
                        exp_ctr += 1
                        exs[mu] = ex
                        if mu >= SKEW:
                            emit_attnv(oh, h, mu - SKEW, exs.pop(mu - SKEW))
                    for mu in range(8 - SKEW, 8):
                        emit_attnv(oh, h, mu, exs.pop(mu))
                    # normalization: per-partition reciprocal + broadcast mult
                    from concourse.dve_ops import (
                        RECIP_APPROX_FAST_CONSTS, RECIPROCAL_APPROX_FAST)
                    _c = RECIP_APPROX_FAST_CONSTS
                    rr = nm.tile([P, 4], F32, tag="rr", name="rr")
                    nc.vector._custom_dve(
                        RECIPROCAL_APPROX_FAST, out=rr[:], in0=oh[:, :, 64],
                        s0=_c["s0"], s1=_c["s1"], imm2=_c["imm2"])
                    rap = rr[:]
                    rr_b = AP(rap.tensor, rap.offset,
                              [rap.ap[0], rap.ap[1], [0, 64]])
                    nc.vector.tensor_tensor(
                        mh[:, ht, nb * 4:(nb + 1) * 4, ab * 64:ab * 64 + 64],
                        oh[:, :, 0:64], rr_b, MULT)
                    if ab == 1:
                        # pair ht complete for this nb: PE-transpose mh pair
                        # tiles into mhT (no crossbar round-trips)
                        for c4 in range(4):
                            nt = nb * 4 + c4
                            psT = ps_pp.tile([P, P], BF16, tag="pp", name="pp")
                            nc.tensor.transpose(
                                psT[:], mh[:, ht, nt, :], ident[:])
                            nc.scalar.copy(mhT[:, ht, nt, :], psT[:])
                if nb == 3:
                    # final block: no later head iteration will flush it
                    emit_out_group(nb)
                else:
                    pending_out.append(nb)

            while pending_out:
                emit_out_group(pending_out.pop())

    nc.compile()
    return nc


def kernel(query, key, value, query_kernel, key_kernel, value_kernel,
           projection_kernel, q_bias, k_bias, v_bias, projection_bias):
    query = np.asarray(query, np.float32)
    key = np.asarray(key, np.float32)
    value = np.asarray(value, np.float32)
    wq = np.asarray(query_kernel, np.float32)
    wk = np.asarray(key_kernel, np.float32)
    wv = np.asarray(value_kernel, np.float32)
    wp = np.asarray(projection_kernel, np.float32)
    qb = np.asarray(q_bias, np.float32)
    kb = np.asarray(k_bias, np.float32)
    vb = np.asarray(v_bias, np.float32)
    pb = np.asarray(projection_bias, np.float32)

    B = query.shape[0]
    const_row = (np.einsum("hi,hio->o", vb.astype(np.float64),
                           wp.astype(np.float64))
                 + pb.astype(np.float64)).astype(np.float32)

    bfq = [np.ascontiguousarray(query[b]).astype(ml_dtypes.bfloat16)
           for b in range(B)]
    bfk = [np.ascontiguousarray(key[b]).astype(ml_dtypes.bfloat16)
           for b in range(B)]
    bfv = [np.ascontiguousarray(value[b]).astype(ml_dtypes.bfloat16)
           for b in range(B)]
    ident = np.eye(P).astype(ml_dtypes.bfloat16)

    halves = []
    for hh in range(2):
        hs = slice(hh * HC, (hh + 1) * HC)
        halves.append(dict(
            wq=np.ascontiguousarray(
                wq[hs].transpose(1, 0, 2).reshape(DM, HDH)).astype(
                ml_dtypes.bfloat16),
            wk=np.ascontiguousarray(
                wk[hs].transpose(1, 0, 2).reshape(DM, HDH)).astype(
                ml_dtypes.bfloat16),
            wv=np.ascontiguousarray(
                wv[hs].transpose(1, 0, 2).reshape(DM, HDH)).astype(
                ml_dtypes.bfloat16),
            wp=np.ascontiguousarray(
                wp[hs].reshape(HDH, DO)).astype(ml_dtypes.bfloat16),
            qb=np.ascontiguousarray(qb[hs].reshape(HDH).reshape(2, P).T),
            kb=np.ascontiguousarray(kb[hs].reshape(HDH).reshape(2, P).T),
            ident=ident,
        ))

    if "nc" not in _CACHED:
        _CACHED["nc"] = _build()
    nc = _CACHED["nc"]

    in_maps = []
    for c in range(8):
        b, hh = c // 2, c % 2
        in_maps.append(dict(q=bfq[b], k=bfk[b], v=bfv[b], **halves[hh]))

    trace = os.environ.get("KERNEL_TRACE", "0") == "1"
    try:
        res = run_bass_kernel_spmd(nc, in_maps, core_ids=list(range(8)),
                                   trace=trace)
    except ModuleNotFoundError:
        res = run_bass_kernel_spmd(nc, in_maps, core_ids=list(range(8)),
                                   trace=False)
    global LAST_EXEC_NS
    LAST_EXEC_NS = res.exec_time_ns
    if trace and res.exec_time_ns is not None:
        print(f"HW exec time: {res.exec_time_ns} ns")
        if res.instructions_and_trace is not None:
            print(f"trace: {res.instructions_and_trace[1]}")

    out = np.empty((B, N, DO), dtype=np.float32)
    for b in range(B):
        out[b] = (res.results[2 * b]["out"] + res.results[2 * b + 1]["out"]
                  + const_row[None, :])
    return out


# revision 39
# speedup vs baseline: 1.4597x; 1.1833x over previous
"""Multi-head attention Trainium2 Bass kernel (v5).

Problem: B=4, N=M=2048, DM=512, H=8, DH=64, DO=512, fp32, rel-err gate 2e-2.

Sharding: 8 cores = (batch b, head-half hh) -- each core computes heads
[4hh, 4hh+4) for all 2048 query rows of one batch, producing a partial
output [2048, 512]; the host sums the two partials per batch and adds the
constant row (sum_h v_bias_h @ Wp_h + proj_bias).

Per-core dataflow (all matmuls bf16/f32r at 1 cycle/row in the cost model):
  - k/q arrive bf16, transposed on load by the DMA crossbar (SP queue only --
    Activation-issued dma_start_transpose returns wrong data on HW);
    loads are split (K halves, Q quarter-first) so projections start early
  - v loaded untransposed in one DMA; transposed on the PE per m-tile
  - qTf/kTf = W^T xT  [hdh-chunk, n] f32r (bias fused into the PSUM copy)
  - vha = xvT^T Wv    [m-tile, 4*65] bf16 (65th col = ones -> softmax sums)
  - scores sc[m-tile, n-block] = kT^T qT per head (f32 PSUM)
  - exp split across engines (interleaved): ScalarE activation Exp
    (scale=1/8) -> bf16; DVE Schraudolph int16 = round(sc*a+b) == bf16 bits
  - attn@V flipped: oh[n-chunk, 65] += ex_chunk^T @ vha[mt, h] (ex is the
    stationary operand; out free size 65; col 64 = softmax denominator);
    oh's 4 sub-bank accumulators share one bank-covering zero-init matmul
  - normalize: rr = fast-recip(oh[:, :, 64]); one broadcast-AP multiply
  - mh pairs transposed on the PE -> mhT; out partial = sum_g mhT_g^T wp2_g
"""
import os
import sys

sys.path.insert(0, "/opt/trn_rl_repo")

import numpy as np
import ml_dtypes

import concourse.bass as bass
import concourse.mybir as mybir
import concourse.tile as tile
from concourse import bacc
from concourse.bass import AP
from concourse.bass_utils import run_bass_kernel_spmd

F32 = mybir.dt.float32
F32R = mybir.dt.float32r
BF16 = mybir.dt.bfloat16
I16 = mybir.dt.int16
EXP = mybir.ActivationFunctionType.Exp
ADD = mybir.AluOpType.add
MULT = mybir.AluOpType.mult

P = 128
DM = 512
HC = 4            # heads per core
DH = 64
HDH = 256         # hdh per core
N = 2048
M = 2048
DO = 512
N_MT = M // P
N_NT = N // P

SCHRAUD_C = 0.0434
A_S = float(np.float32(128.0 / np.log(2.0)) / 8.0)
B_S = float(np.float32(128.0 * (127.0 - SCHRAUD_C)))

_CACHED = {}
LAST_EXEC_NS = None


def _build():
    nc = bacc.Bacc("TRN2", target_bir_lowering=False, debug=False)

    d_q = nc.declare_dram_parameter("q", [N, DM], BF16, isOutput=False)
    d_k = nc.declare_dram_parameter("k", [M, DM], BF16, isOutput=False)
    d_v = nc.declare_dram_parameter("v", [M, DM], BF16, isOutput=False)
    d_wq = nc.declare_dram_parameter("wq", [DM, HDH], BF16, isOutput=False)
    d_wk = nc.declare_dram_parameter("wk", [DM, HDH], BF16, isOutput=False)
    d_wv = nc.declare_dram_parameter("wv", [DM, HDH], BF16, isOutput=False)
    d_wp = nc.declare_dram_parameter("wp", [HDH, DO], BF16, isOutput=False)
    d_qb = nc.declare_dram_parameter("qb", [P, 2], F32, isOutput=False)
    d_kb = nc.declare_dram_parameter("kb", [P, 2], F32, isOutput=False)
    d_id = nc.declare_dram_parameter("ident", [P, P], BF16, isOutput=False)
    d_out = nc.declare_dram_parameter("out", [N, DO], F32, isOutput=True)

    with tile.TileContext(nc) as tc:
        from contextlib import ExitStack
        with ExitStack() as ctx:
            persist = ctx.enter_context(tc.tile_pool(name="persist", bufs=1))
            ex_pool = ctx.enter_context(tc.tile_pool(name="expp", bufs=8))
            vtt_pool = ctx.enter_context(tc.tile_pool(name="vttp", bufs=2))
            nm = ctx.enter_context(tc.tile_pool(name="nm", bufs=2))
            ot_pool = ctx.enter_context(tc.tile_pool(name="otp", bufs=2))
            ps_pp = ctx.enter_context(tc.tile_pool(name="pp", bufs=2, space="PSUM"))
            ps_sc = ctx.enter_context(tc.tile_pool(name="sc", bufs=4, space="PSUM"))
            ps_oh = ctx.enter_context(tc.tile_pool(name="oh", bufs=2, space="PSUM"))

            xkT = persist.tile([P, 4, M], BF16, tag="xkT", name="xkT")
            xqT = persist.tile([P, 4, N], BF16, tag="xqT", name="xqT")
            v_raw = persist.tile([P, N_MT, DM], BF16, tag="v_raw", name="v_raw")
            kTf = persist.tile([P, 2, M], F32R, tag="kTf", name="kTf")
            qTf = persist.tile([P, 2, N], F32R, tag="qTf", name="qTf")
            vha = persist.tile([P, N_MT, HC * 65], BF16, tag="vha", name="vha")
            vhav = vha[:].rearrange("p a (h c) -> p a h c", c=65)
            mh = persist.tile([P, 2, N_NT, P], BF16, tag="mh", name="mh")
            mhT = persist.tile([P, 2, N_NT, P], BF16, tag="mhT", name="mhT")
            wk_sb = persist.tile([P, 4, HDH], BF16, tag="wk", name="wk")
            kb_sb = persist.tile([P, 2], F32, tag="kb", name="kb")
            wq_sb = persist.tile([P, 4, HDH], BF16, tag="wq", name="wq")
            qb_sb = persist.tile([P, 2], F32, tag="qb", name="qb")
            wv_sb = persist.tile([P, 4, HDH], BF16, tag="wv", name="wv")
            wp_sb = persist.tile([P, 2, DO], BF16, tag="wp", name="wp")
            ident = persist.tile([P, P], BF16, tag="ident", name="ident")
            zrow = persist.tile([1, 512], BF16, tag="zrow", name="zrow")

            # all loads on the SP/HWDGE queue in dependency order -- mixing
            # SWDGE (Pool) and HWDGE DMAs serializes them with multi-us
            # round-trips, while a pure HWDGE stream pipelines at ~650ns
            nc.gpsimd.memset(vhav[:, :, :, 64:65], 1.0)
            nc.gpsimd.memset(zrow[:], 0.0)

            def xload(xT, d_x, r0, r1):
                for dc in range(4):
                    nc.sync.dma_start_transpose(
                        xT[:, dc, r0:r1], d_x[r0:r1, dc * P:(dc + 1) * P])

            nc.sync.dma_start(
                wk_sb[:], d_wk[:].rearrange("(a p) c -> p a c", p=P))
            nc.sync.dma_start(kb_sb[:], d_kb[:])
            xload(xkT, d_k, 0, 1024)
            xload(xqT, d_q, 0, 512)
            nc.sync.dma_start(ident[:], d_id[:])
            nc.sync.dma_start(
                wq_sb[:], d_wq[:].rearrange("(a p) c -> p a c", p=P))
            nc.sync.dma_start(qb_sb[:], d_qb[:])
            nc.sync.dma_start(
                wv_sb[:], d_wv[:].rearrange("(a p) c -> p a c", p=P))
            nc.sync.dma_start(
                v_raw[:], d_v[:].rearrange("(a p) c -> p a c", p=P))
            xload(xkT, d_k, 1024, 2048)
            xload(xqT, d_q, 512, 2048)
            nc.sync.dma_start(
                wp_sb[:], d_wp[:].rearrange("(a p) c -> p a c", p=P))

            def proj_block(xT, w_sb, b_sb, dst, mb):
                for ht in range(2):
                    pp = ps_pp.tile([P, 512], F32, tag="pp", name="pp")
                    for dc in range(4):
                        nc.tensor.matmul(
                            pp[:], w_sb[:, dc, ht * P:(ht + 1) * P],
                            xT[:, dc, mb * 512:(mb + 1) * 512],
                            start=(dc == 0), stop=(dc == 3))
                    nc.vector.tensor_scalar(
                        dst[:, ht, mb * 512:(mb + 1) * 512],
                        pp[:], b_sb[:, ht:ht + 1], None, ADD)

            def vproj_tile(mt):
                # PE-transpose the raw v tile, then project into vha
                pst = ps_pp.tile([P, 4, P], BF16, tag="pp", name="pp")
                for dc in range(4):
                    nc.tensor.transpose(
                        pst[:, dc, :], v_raw[:, mt, dc * P:(dc + 1) * P],
                        ident[:])
                vtt = vtt_pool.tile([P, 4, P], BF16, tag="vtt", name="vtt")
                nc.scalar.copy(vtt[:], pst[:])
                pp = ps_pp.tile([P, 512], F32, tag="pp", name="pp")
                for dc in range(4):
                    nc.tensor.matmul(
                        pp[:, 0:HDH], vtt[:, dc, :],
                        wv_sb[:, dc, :], start=(dc == 0), stop=(dc == 3))
                nc.vector.tensor_copy(
                    vhav[:, mt, :, 0:64],
                    pp[:, 0:HDH].rearrange("p (h c) -> p h c", h=HC))

            proj_block(xkT, wk_sb, kb_sb, kTf, 0)
            proj_block(xkT, wk_sb, kb_sb, kTf, 1)
            proj_block(xqT, wq_sb, qb_sb, qTf, 0)
            vproj_tile(0)
            vproj_tile(1)

            # --- attention ---
            exp_ctr = 0
            pending_out = []

            def emit_out_group(nb):
                for c4 in range(4):
                    nt = nb * 4 + c4
                    po = ps_pp.tile([P, DO], F32, tag="pp", name="pp")
                    for g in range(2):
                        nc.tensor.matmul(
                            po[:], mhT[:, g, nt, :], wp_sb[:, g, :],
                            start=(g == 0), stop=(g == 1))
                    ot = ot_pool.tile([P, DO], F32, tag="ot", name="ot")
                    if c4 % 2 == 0:
                        nc.scalar.copy(ot[:], po[:])
                    else:
                        nc.vector.tensor_copy(ot[:], po[:])
                    nc.gpsimd.dma_start(d_out[nt * P:(nt + 1) * P, :], ot[:])

            for nb in range(4):
                for h in range(HC):
                    ht, ab = h // 2, h % 2
                    if h == 2 and nb < 3:
                        proj_block(xqT, wq_sb, qb_sb, qTf, nb + 1)
                    if pending_out and h == 3:
                        emit_out_group(pending_out.pop())
                    oh = ps_oh.tile([P, 4, 65], F32, tag="oh", name="oh")
                    # one bank-covering zero init: sub-bank accumulation
                    # regions must not each issue start=True (the start flag
                    # zero-marks the whole 2KB PSUM bank)
                    nc.tensor.matmul(
                        oh[:].rearrange("p a b -> p (a b)"),
                        zrow[0:1, 0:P], zrow[0:1, 0:260],
                        start=True, stop=False, skip_group_check=True)

                    def emit_attnv(oh, h, mt, exd):
                        for c4 in range(4):
                            nc.tensor.matmul(
                                oh[:, c4, :],
                                exd[:, c4 * P:(c4 + 1) * P].bitcast(BF16),
                                vhav[:, mt, h, :],
                                start=False,
                                stop=(mt == 15),
                                skip_group_check=True)

                    SKEW = 5
                    exs = {}
                    for mt in range(16):
                        mu = mt // 2
                        if nb == 0 and h == 0 and mt < 14:
                            if mt == 2:
                                proj_block(xkT, wk_sb, kb_sb, kTf, 2)
                            if mt == 4:
                                proj_block(xkT, wk_sb, kb_sb, kTf, 3)
                            if mt % 2 == 0:
                                vproj_tile(mt + 2)
                                vproj_tile(mt + 3)
                        sc = ps_sc.tile([P, 512], F32, tag="sc", name="sc")
                        nc.tensor.matmul(
                            sc[:],
                            kTf[ab * 64:ab * 64 + 64, ht, mt * P:(mt + 1) * P],
                            qTf[ab * 64:ab * 64 + 64, ht,
                                nb * 512:(nb + 1) * 512],
                            start=True, stop=True)
                        ex = ex_pool.tile([P, 512], I16, tag="ex", name="ex")
                        if exp_ctr % 16 in (1, 3, 5, 7, 9, 11, 13):
                            nc.vector.tensor_scalar(
                                ex[:], sc[:], A_S, B_S, MULT, ADD)
                        else:
                            nc.scalar.activation(
                                ex[:].bitcast(BF16), sc[:], EXP, scale=0.125)
                        exp_ctr += 1
                        exs[mt] = ex
                        if mt >= SKEW:
                            emit_attnv(oh, h, mt - SKEW, exs.pop(mt - SKEW))
                    for mt in range(16 - SKEW, 16):
                        emit_attnv(oh, h, mt, exs.pop(mt))
                    # normalization: per-partition reciprocal + broadcast mult
                    from concourse.dve_ops import (
                        RECIP_APPROX_FAST_CONSTS, RECIPROCAL_APPROX_FAST)
                    _c = RECIP_APPROX_FAST_CONSTS
                    rr = nm.tile([P, 4], F32, tag="rr", name="rr")
                    nc.vector._custom_dve(
                        RECIPROCAL_APPROX_FAST, out=rr[:], in0=oh[:, :, 64],
                        s0=_c["s0"], s1=_c["s1"], imm2=_c["imm2"])
                    rap = rr[:]
                    rr_b = AP(rap.tensor, rap.offset,
                              [rap.ap[0], rap.ap[1], [0, 64]])
                    nc.vector.tensor_tensor(
                        mh[:, ht, nb * 4:(nb + 1) * 4, ab * 64:ab * 64 + 64],
                        oh[:, :, 0:64], rr_b, MULT)
                    if ab == 1:
                        # pair ht complete for this nb: PE-transpose mh pair
                        # tiles into mhT (no crossbar round-trips)
                        for c4 in range(4):
                            nt = nb * 4 + c4
                            psT = ps_pp.tile([P, P], BF16, tag="pp", name="pp")
                            nc.tensor.transpose(
                                psT[:], mh[:, ht, nt, :], ident[:])
                            nc.scalar.copy(mhT[:, ht, nt, :], psT[:])
                if nb == 3:
                    # final block: no later head iteration will flush it
                    emit_out_group(nb)
                else:
                    pending_out.append(nb)

            while pending_out:
                emit_out_group(pending_out.pop())

    nc.compile()
    return nc


def kernel(query, key, value, query_kernel, key_kernel, value_kernel,
           projection_kernel, q_bias, k_bias, v_bias, projection_bias):
    query = np.asarray(query, np.float32)
    key = np.asarray(key, np.float32)
    value = np.asarray(value, np.float32)
    wq = np.asarray(query_kernel, np.float32)
    wk = np.asarray(key_kernel, np.float32)
    wv = np.asarray(value_kernel, np.float32)
    wp = np.asarray(projection_kernel, np.float32)
    qb = np.asarray(q_bias, np.float32)
    kb = np.asarray(k_bias, np.float32)
    vb = np.asarray(v_bias, np.float32)
    pb = np.asarray(projection_bias, np.float32)

    B = query.shape[0]
    const_row = (np.einsum("hi,hio->o", vb.astype(np.float64),
                           wp.astype(np.float64))
                 + pb.astype(np.float64)).astype(np.float32)

    bfq = [np.ascontiguousarray(query[b]).astype(ml_dtypes.bfloat16)
           for b in range(B)]
    bfk = [np.ascontiguousarray(key[b]).astype(ml_dtypes.bfloat16)
           for b in range(B)]
    bfv = [np.ascontiguousarray(value[b]).astype(ml_dtypes.bfloat16)
           for b in range(B)]
    ident = np.eye(P).astype(ml_dtypes.bfloat16)

    halves = []
    for hh in range(2):
        hs = slice(hh * HC, (hh + 1) * HC)
        halves.append(dict(
            wq=np.ascontiguousarray(
                wq[hs].transpose(1, 0, 2).reshape(DM, HDH)).astype(
                ml_dtypes.bfloat16),
            wk=np.ascontiguousarray(
                wk[hs].transpose(1, 0, 2).reshape(DM, HDH)).astype(
                ml_dtypes.bfloat16),
            wv=np.ascontiguousarray(
                wv[hs].transpose(1, 0, 2).reshape(DM, HDH)).astype(
                ml_dtypes.bfloat16),
            wp=np.ascontiguousarray(
                wp[hs].reshape(HDH, DO)).astype(ml_dtypes.bfloat16),
            qb=np.ascontiguousarray(qb[hs].reshape(HDH).reshape(2, P).T),
            kb=np.ascontiguousarray(kb[hs].reshape(HDH).reshape(2, P).T),
            ident=ident,
        ))

    if "nc" not in _CACHED:
        _CACHED["nc"] = _build()
    nc = _CACHED["nc"]

    in_maps = []
    for c in range(8):
        b, hh = c // 2, c % 2
        in_maps.append(dict(q=bfq[b], k=bfk[b], v=bfv[b], **halves[hh]))

    trace = os.environ.get("KERNEL_TRACE", "0") == "1"
    try:
        res = run_bass_kernel_spmd(nc, in_maps, core_ids=list(range(8)),
                                   trace=trace)
    except ModuleNotFoundError:
        res = run_bass_kernel_spmd(nc, in_maps, core_ids=list(range(8)),
                                   trace=False)
    global LAST_EXEC_NS
    LAST_EXEC_NS = res.exec_time_ns
    if trace and res.exec_time_ns is not None:
        print(f"HW exec time: {res.exec_time_ns} ns")
        if res.instructions_and_trace is not None:
            print(f"trace: {res.instructions_and_trace[1]}")

    out = np.empty((B, N, DO), dtype=np.float32)
    for b in range(B):
        out[b] = (res.results[2 * b]["out"] + res.results[2 * b + 1]["out"]
                  + const_row[None, :])
    return out


# revision 51
# speedup vs baseline: 1.4904x; 1.0211x over previous
"""Multi-head attention Trainium2 Bass kernel (v5).

Problem: B=4, N=M=2048, DM=512, H=8, DH=64, DO=512, fp32, rel-err gate 2e-2.

Sharding: 8 cores = (batch b, head-half hh) -- each core computes heads
[4hh, 4hh+4) for all 2048 query rows of one batch, producing a partial
output [2048, 512]; the host sums the two partials per batch and adds the
constant row (sum_h v_bias_h @ Wp_h + proj_bias).

Per-core dataflow (all matmuls bf16/f32r at 1 cycle/row in the cost model):
  - k/q arrive bf16, transposed on load by the DMA crossbar (SP queue only --
    Activation-issued dma_start_transpose returns wrong data on HW);
    loads are split (K halves, Q quarter-first) so projections start early
  - v loaded untransposed in one DMA; transposed on the PE per m-tile
  - qTf/kTf = W^T xT  [hdh-chunk, n] f32r (bias fused into the PSUM copy)
  - vha = xvT^T Wv    [m-tile, 4*65] bf16 (65th col = ones -> softmax sums)
  - scores sc[m-tile, n-block] = kT^T qT per head (f32 PSUM)
  - exp split across engines (interleaved): ScalarE activation Exp
    (scale=1/8) -> bf16; DVE Schraudolph int16 = round(sc*a+b) == bf16 bits
  - attn@V flipped: oh[n-chunk, 65] += ex_chunk^T @ vha[mt, h] (ex is the
    stationary operand; out free size 65; col 64 = softmax denominator);
    oh's 4 sub-bank accumulators share one bank-covering zero-init matmul
  - normalize: rr = fast-recip(oh[:, :, 64]); one broadcast-AP multiply
  - mh pairs transposed on the PE -> mhT; out partial = sum_g mhT_g^T wp2_g
"""
import os
import sys

sys.path.insert(0, "/opt/trn_rl_repo")

import numpy as np
import ml_dtypes

import concourse.bass as bass
import concourse.mybir as mybir
import concourse.tile as tile
from concourse import bacc
from concourse.bass import AP
from concourse.bass_utils import run_bass_kernel_spmd

F32 = mybir.dt.float32
F32R = mybir.dt.float32r
BF16 = mybir.dt.bfloat16
I16 = mybir.dt.int16
EXP = mybir.ActivationFunctionType.Exp
ADD = mybir.AluOpType.add
MULT = mybir.AluOpType.mult

P = 128
DM = 512
HC = 4            # heads per core
DH = 64
HDH = 256         # hdh per core
N = 2048
M = 2048
DO = 512
N_MT = M // P
N_NT = N // P

SCHRAUD_C = 0.0434
A_S = float(np.float32(128.0 / np.log(2.0)) / 8.0)
B_S = float(np.float32(128.0 * (127.0 - SCHRAUD_C)))

_CACHED = {}
LAST_EXEC_NS = None


def _build():
    nc = bacc.Bacc("TRN2", target_bir_lowering=False, debug=False)

    d_q = nc.declare_dram_parameter("q", [N, DM], BF16, isOutput=False)
    d_k = nc.declare_dram_parameter("k", [M, DM], BF16, isOutput=False)
    d_v = nc.declare_dram_parameter("v", [M, DM], BF16, isOutput=False)
    d_wq = nc.declare_dram_parameter("wq", [DM, HDH], BF16, isOutput=False)
    d_wk = nc.declare_dram_parameter("wk", [DM, HDH], BF16, isOutput=False)
    d_wv = nc.declare_dram_parameter("wv", [DM, HDH], BF16, isOutput=False)
    d_wp = nc.declare_dram_parameter("wp", [HDH, DO], BF16, isOutput=False)
    d_qb = nc.declare_dram_parameter("qb", [P, 2], F32, isOutput=False)
    d_kb = nc.declare_dram_parameter("kb", [P, 2], F32, isOutput=False)
    d_id = nc.declare_dram_parameter("ident", [P, P], BF16, isOutput=False)
    d_out = nc.declare_dram_parameter("out", [N, DO], F32, isOutput=True)

    with tile.TileContext(nc) as tc:
        from contextlib import ExitStack
        with ExitStack() as ctx:
            persist = ctx.enter_context(tc.tile_pool(name="persist", bufs=1))
            ex_pool = ctx.enter_context(tc.tile_pool(name="expp", bufs=8))
            vtt_pool = ctx.enter_context(tc.tile_pool(name="vttp", bufs=2))
            nm = ctx.enter_context(tc.tile_pool(name="nm", bufs=2))
            ot_pool = ctx.enter_context(tc.tile_pool(name="otp", bufs=2))
            ps_pp = ctx.enter_context(tc.tile_pool(name="pp", bufs=2, space="PSUM"))
            ps_sc = ctx.enter_context(tc.tile_pool(name="sc", bufs=4, space="PSUM"))
            ps_oh = ctx.enter_context(tc.tile_pool(name="oh", bufs=2, space="PSUM"))

            xkT = persist.tile([P, 4, M], BF16, tag="xkT", name="xkT")
            xqT = persist.tile([P, 4, N], BF16, tag="xqT", name="xqT")
            v_raw = persist.tile([P, N_MT, DM], BF16, tag="v_raw", name="v_raw")
            kTf = persist.tile([P, 2, M], F32R, tag="kTf", name="kTf")
            qTf = persist.tile([P, 2, N], F32R, tag="qTf", name="qTf")
            vha = persist.tile([P, N_MT, HC * 65], BF16, tag="vha", name="vha")
            vhav = vha[:].rearrange("p a (h c) -> p a h c", c=65)
            mh = persist.tile([P, 2, N_NT, P], BF16, tag="mh", name="mh")
            mhT = persist.tile([P, 2, N_NT, P], BF16, tag="mhT", name="mhT")
            wk_sb = persist.tile([P, 4, HDH], BF16, tag="wk", name="wk")
            kb_sb = persist.tile([P, 2], F32, tag="kb", name="kb")
            wq_sb = persist.tile([P, 4, HDH], BF16, tag="wq", name="wq")
            qb_sb = persist.tile([P, 2], F32, tag="qb", name="qb")
            wv_sb = persist.tile([P, 4, HDH], BF16, tag="wv", name="wv")
            wp_sb = persist.tile([P, 2, DO], BF16, tag="wp", name="wp")
            ident = persist.tile([P, P], BF16, tag="ident", name="ident")
            zrow = persist.tile([1, 512], BF16, tag="zrow", name="zrow")

            # all loads on the SP/HWDGE queue in dependency order -- mixing
            # SWDGE (Pool) and HWDGE DMAs serializes them with multi-us
            # round-trips, while a pure HWDGE stream pipelines at ~650ns
            nc.gpsimd.memset(vhav[:, :, :, 64:65], 1.0)
            nc.gpsimd.memset(zrow[:], 0.0)

            def xload(xT, d_x, r0, r1):
                for dc in range(4):
                    nc.sync.dma_start_transpose(
                        xT[:, dc, r0:r1], d_x[r0:r1, dc * P:(dc + 1) * P])

            nc.sync.dma_start(
                wk_sb[:], d_wk[:].rearrange("(a p) c -> p a c", p=P))
            nc.sync.dma_start(kb_sb[:], d_kb[:])
            nc.sync.dma_start(
                wq_sb[:], d_wq[:].rearrange("(a p) c -> p a c", p=P))
            nc.sync.dma_start(qb_sb[:], d_qb[:])
            xload(xkT, d_k, 0, 1024)
            xload(xqT, d_q, 0, 512)
            nc.sync.dma_start(ident[:], d_id[:])
            nc.sync.dma_start(
                wv_sb[:], d_wv[:].rearrange("(a p) c -> p a c", p=P))
            nc.sync.dma_start(
                v_raw[:], d_v[:].rearrange("(a p) c -> p a c", p=P))
            xload(xkT, d_k, 1024, 2048)
            xload(xqT, d_q, 512, 2048)
            nc.sync.dma_start(
                wp_sb[:], d_wp[:].rearrange("(a p) c -> p a c", p=P))

            def proj_block(xT, w_sb, b_sb, dst, mb):
                for ht in range(2):
                    pp = ps_pp.tile([P, 512], F32, tag="pp", name="pp")
                    for dc in range(4):
                        nc.tensor.matmul(
                            pp[:], w_sb[:, dc, ht * P:(ht + 1) * P],
                            xT[:, dc, mb * 512:(mb + 1) * 512],
                            start=(dc == 0), stop=(dc == 3))
                    nc.vector.tensor_scalar(
                        dst[:, ht, mb * 512:(mb + 1) * 512],
                        pp[:], b_sb[:, ht:ht + 1], None, ADD)

            def vproj_tile(mt):
                # PE-transpose the raw v tile, then project into vha
                pst = ps_pp.tile([P, 4, P], BF16, tag="pp", name="pp")
                for dc in range(4):
                    nc.tensor.transpose(
                        pst[:, dc, :], v_raw[:, mt, dc * P:(dc + 1) * P],
                        ident[:])
                vtt = vtt_pool.tile([P, 4, P], BF16, tag="vtt", name="vtt")
                if mt % 2 == 0:
                    nc.scalar.copy(vtt[:], pst[:])
                else:
                    nc.vector.tensor_copy(vtt[:], pst[:])
                pp = ps_pp.tile([P, 512], F32, tag="pp", name="pp")
                for dc in range(4):
                    nc.tensor.matmul(
                        pp[:, 0:HDH], vtt[:, dc, :],
                        wv_sb[:, dc, :], start=(dc == 0), stop=(dc == 3))
                nc.vector.tensor_copy(
                    vhav[:, mt, :, 0:64],
                    pp[:, 0:HDH].rearrange("p (h c) -> p h c", h=HC))

            proj_block(xkT, wk_sb, kb_sb, kTf, 0)
            proj_block(xkT, wk_sb, kb_sb, kTf, 1)
            proj_block(xqT, wq_sb, qb_sb, qTf, 0)
            vproj_tile(0)
            vproj_tile(1)

            # --- attention ---
            exp_ctr = 0
            pending_out = []
            pending_tr = []

            def emit_out_group(nb):
                for c4 in range(4):
                    nt = nb * 4 + c4
                    po = ps_pp.tile([P, DO], F32, tag="pp", name="pp")
                    for g in range(2):
                        nc.tensor.matmul(
                            po[:], mhT[:, g, nt, :], wp_sb[:, g, :],
                            start=(g == 0), stop=(g == 1))
                    ot = ot_pool.tile([P, DO], F32, tag="ot", name="ot")
                    if c4 % 2 == 0:
                        nc.scalar.copy(ot[:], po[:])
                    else:
                        nc.vector.tensor_copy(ot[:], po[:])
                    nc.gpsimd.dma_start(d_out[nt * P:(nt + 1) * P, :], ot[:])

            for nb in range(4):
                for h in range(HC):
                    ht, ab = h // 2, h % 2
                    if h == 2 and nb < 3:
                        proj_block(xqT, wq_sb, qb_sb, qTf, nb + 1)
                    if pending_out and h == 3:
                        emit_out_group(pending_out.pop())
                    oh = ps_oh.tile([P, 4, 65], F32, tag="oh", name="oh")
                    # one bank-covering zero init: sub-bank accumulation
                    # regions must not each issue start=True (the start flag
                    # zero-marks the whole 2KB PSUM bank)
                    nc.tensor.matmul(
                        oh[:].rearrange("p a b -> p (a b)"),
                        zrow[0:1, 0:P], zrow[0:1, 0:260],
                        start=True, stop=False, skip_group_check=True)

                    def emit_attnv(oh, h, mt, exd):
                        for c4 in range(4):
                            nc.tensor.matmul(
                                oh[:, c4, :],
                                exd[:, c4 * P:(c4 + 1) * P].bitcast(BF16),
                                vhav[:, mt, h, :],
                                start=False,
                                stop=(mt == 15),
                                skip_group_check=True)

                    SKEW = 3 if (nb == 3 and h == 3) else 5
                    exs = {}
                    for mt in range(16):
                        mu = mt // 2
                        if nb == 0 and h == 0 and mt < 14:
                            if mt == 2:
                                proj_block(xkT, wk_sb, kb_sb, kTf, 2)
                            if mt == 4:
                                proj_block(xkT, wk_sb, kb_sb, kTf, 3)
                            if mt % 2 == 0:
                                vproj_tile(mt + 2)
                                vproj_tile(mt + 3)
                        sc = ps_sc.tile([P, 512], F32, tag="sc", name="sc")
                        nc.tensor.matmul(
                            sc[:],
                            kTf[ab * 64:ab * 64 + 64, ht, mt * P:(mt + 1) * P],
                            qTf[ab * 64:ab * 64 + 64, ht,
                                nb * 512:(nb + 1) * 512],
                            start=True, stop=True)
                        ex = ex_pool.tile([P, 512], I16, tag="ex", name="ex")
                        if exp_ctr % 16 in (1, 3, 5, 7, 9, 11, 13):
                            nc.vector.tensor_scalar(
                                ex[:], sc[:], A_S, B_S, MULT, ADD)
                        else:
                            nc.scalar.activation(
                                ex[:].bitcast(BF16), sc[:], EXP, scale=0.125)
                        exp_ctr += 1
                        exs[mt] = ex
                        if mt >= SKEW:
                            emit_attnv(oh, h, mt - SKEW, exs.pop(mt - SKEW))
                    for mt in range(16 - SKEW, 16):
                        emit_attnv(oh, h, mt, exs.pop(mt))
                    # normalization: per-partition reciprocal + broadcast mult
                    from concourse.dve_ops import (
                        RECIP_APPROX_FAST_CONSTS, RECIPROCAL_APPROX_FAST)
                    _c = RECIP_APPROX_FAST_CONSTS
                    rr = nm.tile([P, 4], F32, tag="rr", name="rr")
                    nc.vector._custom_dve(
                        RECIPROCAL_APPROX_FAST, out=rr[:], in0=oh[:, :, 64],
                        s0=_c["s0"], s1=_c["s1"], imm2=_c["imm2"])
                    rap = rr[:]
                    rr_b = AP(rap.tensor, rap.offset,
                              [rap.ap[0], rap.ap[1], [0, 64]])
                    nc.vector.tensor_tensor(
                        mh[:, ht, nb * 4:(nb + 1) * 4, ab * 64:ab * 64 + 64],
                        oh[:, :, 0:64], rr_b, MULT)
                    if ab == 1:
                        pending_tr.append((ht, nb))
                if nb == 3:
                    # final block: flush any deferred transposes then emit
                    while pending_tr:
                        tr_ht, tr_nb = pending_tr.pop(0)
                        for c4 in range(4):
                            nt = tr_nb * 4 + c4
                            psT = ps_pp.tile([P, P], BF16, tag="pp", name="pp")
                            nc.tensor.transpose(
                                psT[:], mh[:, tr_ht, nt, :], ident[:])
                            nc.scalar.copy(mhT[:, tr_ht, nt, :], psT[:])
                    emit_out_group(nb)
                else:
                    pending_out.append(nb)

            while pending_out:
                emit_out_group(pending_out.pop())

    nc.compile()
    return nc


def kernel(query, key, value, query_kernel, key_kernel, value_kernel,
           projection_kernel, q_bias, k_bias, v_bias, projection_bias):
    query = np.asarray(query, np.float32)
    key = np.asarray(key, np.float32)
    value = np.asarray(value, np.float32)
    wq = np.asarray(query_kernel, np.float32)
    wk = np.asarray(key_kernel, np.float32)
    wv = np.asarray(value_kernel, np.float32)
    wp = np.asarray(projection_kernel, np.float32)
    qb = np.asarray(q_bias, np.float32)
    kb = np.asarray(k_bias, np.float32)
    vb = np.asarray(v_bias, np.float32)
    pb = np.asarray(projection_bias, np.float32)

    B = query.shape[0]
    const_row = (np.einsum("hi,hio->o", vb.astype(np.float64),
                           wp.astype(np.float64))
                 + pb.astype(np.float64)).astype(np.float32)

    bfq = [np.ascontiguousarray(query[b]).astype(ml_dtypes.bfloat16)
           for b in range(B)]
    bfk = [np.ascontiguousarray(key[b]).astype(ml_dtypes.bfloat16)
           for b in range(B)]
    bfv = [np.ascontiguousarray(value[b]).astype(ml_dtypes.bfloat16)
           for b in range(B)]
    ident = np.eye(P).astype(ml_dtypes.bfloat16)

    halves = []
    for hh in range(2):
        hs = slice(hh * HC, (hh + 1) * HC)
        halves.append(dict(
            wq=np.ascontiguousarray(
                wq[hs].transpose(1, 0, 2).reshape(DM, HDH)).astype(
                ml_dtypes.bfloat16),
            wk=np.ascontiguousarray(
                wk[hs].transpose(1, 0, 2).reshape(DM, HDH)).astype(
                ml_dtypes.bfloat16),
            wv=np.ascontiguousarray(
                wv[hs].transpose(1, 0, 2).reshape(DM, HDH)).astype(
                ml_dtypes.bfloat16),
            wp=np.ascontiguousarray(
                wp[hs].reshape(HDH, DO)).astype(ml_dtypes.bfloat16),
            qb=np.ascontiguousarray(qb[hs].reshape(HDH).reshape(2, P).T),
            kb=np.ascontiguousarray(kb[hs].reshape(HDH).reshape(2, P).T),
            ident=ident,
        ))

    if "nc" not in _CACHED:
        _CACHED["nc"] = _build()
    nc = _CACHED["nc"]

    in_maps = []
    for c in range(8):
        b, hh = c // 2, c % 2
        in_maps.append(dict(q=bfq[b], k=bfk[b], v=bfv[b], **halves[hh]))

    trace = os.environ.get("KERNEL_TRACE", "0") == "1"
    try:
        res = run_bass_kernel_spmd(nc, in_maps, core_ids=list(range(8)),
                                   trace=trace)
    except ModuleNotFoundError:
        res = run_bass_kernel_spmd(nc, in_maps, core_ids=list(range(8)),
                                   trace=False)
    global LAST_EXEC_NS
    LAST_EXEC_NS = res.exec_time_ns
    if trace and res.exec_time_ns is not None:
        print(f"HW exec time: {res.exec_time_ns} ns")
        if res.instructions_and_trace is not None:
            print(f"trace: {res.instructions_and_trace[1]}")

    out = np.empty((B, N, DO), dtype=np.float32)
    for b in range(B):
        out[b] = (res.results[2 * b]["out"] + res.results[2 * b + 1]["out"]
                  + const_row[None, :])
    return out
